# revision 31
# baseline (speedup 1.0000x reference)
"""Trainium2 Bass kernel for per-batch channel attention (CxAM-style).

Reference (per batch element b):
    q = (Wq @ x_b + bq)        # [64, T]
    k = (Wk @ x_b + bk)        # [64, T]
    v = (Wv @ x_b + bv)        # [512, T]
    R = q.T @ k                # [T, T]
    A = softmax(R, axis=-1)
    out_b = v @ A.T            # [512, T]

Sharding: pure data-parallel — batch B=8, one batch element per NeuronCore.

Per-core algorithm (layouts chosen so no attention-matrix transposes are
needed and every heavy matmul has free dim 512 in bf16 => full PE rate):
    QK   [128, T] bf16   rows 0:64 = Q, 64:128 = K  (packed projection)
    VT   [s=128 x 16, c=512] bf16 = x.T @ Wv.T + bv (V transposed, bias in)
    per t-block of 512:
      ST_j [s=128, t=512] = K_chunk.T @ Q_block      (scores, transposed;
            row-packed pairs run concurrently on the two PE half-arrays)
      E_j = exp(ST_j)  (bf16; no max needed: |R| <= ~11)
      denom partials: ones-matmuls 4-way COLUMN-TILED (tile_position
            (0,32k)) so 4 of them run concurrently on the PE array; the
            partials land on partitions {0,32,64,96} of one PSUM bank and
            are summed + broadcast to 128 partitions by a single
            ones-stationary matmul.
      U_ck [c=128, t] += VT_chunk_ck.T @ E_j         (unnormalized out)
      out[ck, t] = U_ck * reciprocal(denom broadcast)
Pipelining: x streams in 16 quarter-chunks with the QK projection
accumulating incrementally as they land; the V projection interleaves
with the first t-block's score matmuls; the main loop runs scores with a
lookahead of 8 pairs so every E tile of a t-block exists before its
consume phase starts, letting the whole denominator/reciprocal chain
hide under the AV matmuls.
"""

import os

os.environ.setdefault("MYCRO_LOCAL_CACHE", "1")

import numpy as np

import concourse.bass as bass
import concourse.mybir as mybir
import concourse.tile as tile
from concourse import bacc
from concourse.bass_utils import run_bass_kernel_spmd

F32 = mybir.dt.float32
BF16 = mybir.dt.bfloat16
AF = mybir.ActivationFunctionType

B = 8
C = 512
T = 2048
CQ = 64
NCORES = 8

TB = 512            # t-block (free dim of main matmuls)
NTB = T // TB       # 4
NSC = T // 128      # 16 s-chunks
NPAIR = NSC // 2    # 8 row-packed score pairs per t-block
NCH = C // 128      # 4 contraction chunks
NCC = C // 128      # 4 output channel chunks
LOOKAHEAD = NPAIR   # scores emitted this many pairs ahead of consumes


def _build_program() -> bass.Bass:
    nc = bacc.Bacc("TRN2", target_bir_lowering=False, debug=False, num_devices=1)

    x_d = nc.declare_dram_parameter("x", [C, T], F32, isOutput=False)
    wq_d = nc.declare_dram_parameter("Wq", [CQ, C], F32, isOutput=False)
    bq_d = nc.declare_dram_parameter("bq", [CQ, 1], F32, isOutput=False)
    wk_d = nc.declare_dram_parameter("Wk", [CQ, C], F32, isOutput=False)
    bk_d = nc.declare_dram_parameter("bk", [CQ, 1], F32, isOutput=False)
    wv_d = nc.declare_dram_parameter("Wv", [C, C], F32, isOutput=False)
    bv_d = nc.declare_dram_parameter("bv", [1, C], F32, isOutput=False)
    id_d = nc.declare_dram_parameter("ident", [128, 128], F32, isOutput=False)
    out_d = nc.declare_dram_parameter("out", [C, T], F32, isOutput=True)

    with tile.TileContext(nc) as tc:
        with (
            tc.tile_pool(name="const", bufs=1) as const,
            tc.tile_pool(name="weights", bufs=1) as wpool,
        ):
            ones128b = const.tile([128, 128], BF16)
            nc.vector.memset(ones128b[:], 1.0)
            ident = const.tile([128, 128], F32)
            nc.sync.dma_start(out=ident[:], in_=id_d[:])
            ones_col = const.tile([128, 1], BF16)
            nc.vector.memset(ones_col[:], 1.0)
            ones_row_bf = const.tile([1, 128], BF16)
            nc.vector.memset(ones_row_bf[:], 1.0)
            # staging tile for the 4 col-tiled denominator partials; only
            # partitions {0,32,64,96} are ever written, the rest stay zero
            # so a ones-stationary matmul over all 128 partitions sums
            # exactly the 4 partials (and broadcasts the sum).
            d4sb = const.tile([128, TB], BF16)
            nc.vector.memset(d4sb[:], 0.0)

            # ---- HAM warm-up: the PE clock gate defaults to 1.2 GHz and
            # only ungates after ~3.4us of sustained matmul activity.  Burn
            # that window on dummy matmuls while the input DMAs are still in
            # flight so the real head matmuls run at 2.4 GHz.
            with tc.tile_pool(name="warm", bufs=1, space="PSUM") as wu:
                wt = wu.tile([128, 128], F32, tag="wu")
                for _ in range(12):
                    nc.tensor.matmul(
                        wt[:], ones128b[:], ones128b[:], start=True, stop=True
                    )

            # ---- input DMAs: small weights, then Wv, then x in 16
            # quarter-chunks (quarter-major so the QK projection for a
            # t-quarter can finish as soon as its 4 channel chunks land)
            wq_s = wpool.tile([CQ, C], F32)
            nc.sync.dma_start(out=wq_s[:], in_=wq_d[:])
            wk_s = wpool.tile([CQ, C], F32)
            nc.sync.dma_start(out=wk_s[:], in_=wk_d[:])
            bqk = wpool.tile([128, 1], F32)
            nc.sync.dma_start(out=bqk[0:CQ, :], in_=bq_d[:])
            nc.sync.dma_start(out=bqk[CQ:128, :], in_=bk_d[:])
            bv_row = wpool.tile([1, C], F32)
            nc.sync.dma_start(out=bv_row[:], in_=bv_d[:])
            bv_row_bf = wpool.tile([1, C], BF16)
            nc.vector.tensor_copy(bv_row_bf[:], bv_row[:])
            wv_s = wpool.tile([128, NCH, C], F32)
            wv_r = wv_d[:].rearrange("(po pi) c -> pi po c", pi=128)
            for po in range(NCH):
                nc.sync.dma_start(out=wv_s[:, po, :], in_=wv_r[:, po, :])

            x_s = wpool.tile([128, NCH, T], F32)
            x_bf = wpool.tile([128, NCH, T], BF16)
            x_r = x_d[:].rearrange("(po pi) t -> pi po t", pi=128)
            nq = 0
            for q in range(NTB):
                qsl = slice(q * TB, (q + 1) * TB)
                for ci in range(NCH):
                    nc.sync.dma_start(out=x_s[:, ci, qsl], in_=x_r[:, ci, qsl])
                    if nq % 2 == 0:
                        nc.vector.tensor_copy(x_bf[:, ci, qsl], x_s[:, ci, qsl])
                    else:
                        nc.scalar.activation(x_bf[:, ci, qsl], x_s[:, ci, qsl], AF.Copy)
                    nq += 1

            # ---- transpose weights on PE (runs in the DMA shadow).
            # bf16 inputs: transpose-mode matmuls run 1 cycle/row in bf16 vs
            # 2 for fp32, and the DVE casts hide under the DMAs.
            wq_bf = wpool.tile([CQ, C], BF16)
            nc.vector.tensor_copy(wq_bf[:], wq_s[:])
            wk_bf = wpool.tile([CQ, C], BF16)
            nc.vector.tensor_copy(wk_bf[:], wk_s[:])
            ident_bf = wpool.tile([128, 128], BF16)
            nc.vector.tensor_copy(ident_bf[:], ident[:])
            wv_bf = wpool.tile([128, NCH, C], BF16)
            for po in range(NCH):
                nc.vector.tensor_copy(wv_bf[:, po, :], wv_s[:, po, :])

            wqkT = wpool.tile([128, NCH, 128], BF16)  # [ch, chunk, 0:64 WqT | 64:128 WkT]
            wvT = wpool.tile([128, NCH, C], BF16)     # [ch, chunk, c]
            with tc.tile_pool(name="psum_t", bufs=2, space="PSUM") as pt:
                for j in range(NCH):
                    ptq = pt.tile([128, CQ], BF16, tag="pt")
                    nc.tensor.transpose(
                        ptq[:], wq_bf[:, j * 128:(j + 1) * 128], ident_bf[0:CQ, 0:CQ]
                    )
                    nc.vector.tensor_copy(wqkT[:, j, 0:CQ], ptq[:])
                    ptk = pt.tile([128, CQ], BF16, tag="pt")
                    nc.tensor.transpose(
                        ptk[:], wk_bf[:, j * 128:(j + 1) * 128], ident_bf[0:CQ, 0:CQ]
                    )
                    nc.vector.tensor_copy(wqkT[:, j, CQ:128], ptk[:])
                for i in range(NCH):       # c chunk of Wv rows
                    for j in range(NCH):   # ch chunk of Wv cols
                        ptv = pt.tile([128, 128], BF16, tag="pt")
                        nc.tensor.transpose(
                            ptv[:], wv_bf[:, i, j * 128:(j + 1) * 128], ident_bf[:]
                        )
                        nc.vector.tensor_copy(
                            wvT[:, j, i * 128:(i + 1) * 128], ptv[:]
                        )

            qk = wpool.tile([128, T], BF16)   # rows 0:64 Q, 64:128 K
            kq = wpool.tile([128, T], BF16)   # rows 0:64 K, 64:128 Q
            vT = wpool.tile([128, NSC, C], BF16)
            bv_bcast = wpool.tile([128, C], F32)

            with (
                tc.tile_pool(name="et", bufs=LOOKAHEAD + 2) as et_pool,
                tc.tile_pool(name="ps_sc", bufs=1, space="PSUM") as ps_sc,
            ):
                etp_of = {}

                def emit_scores(tb, jp):
                    tsl = slice(tb * TB, (tb + 1) * TB)
                    j0, j1 = 2 * jp, 2 * jp + 1
                    etp = et_pool.tile(
                        [128, 2, TB], BF16, tag="etp", name=f"etp_{tb}_{jp}"
                    )
                    sc0 = ps_sc.tile([128, TB], F32, tag="sc0", name=f"sc0_{tb}_{jp}")
                    nc.tensor.matmul(
                        sc0[:],
                        kq[0:CQ, j0 * 128:(j0 + 1) * 128],
                        qk[0:CQ, tsl],
                        start=True,
                        stop=True,
                    )
                    sc1 = ps_sc.tile([128, TB], F32, tag="sc1", name=f"sc1_{tb}_{jp}")
                    nc.tensor.matmul(
                        sc1[:],
                        qk[CQ:128, j1 * 128:(j1 + 1) * 128],
                        kq[CQ:128, tsl],
                        start=True,
                        stop=True,
                        tile_position=(64, 0),
                    )
                    nc.scalar.activation(etp[:, 0, :], sc0[:], AF.Exp)
                    nc.scalar.activation(etp[:, 1, :], sc1[:], AF.Exp)
                    etp_of[(tb, jp)] = etp

                # ---- per-quarter head: the QK projection for quarter 0,
                # then per quarter: the V^T projection for its 4 s-chunks,
                # the first t-block's score pairs for those s-chunks, and
                # the next quarter's QK projection — everything streams with
                # the x quarter-chunk DMAs.
                with tc.tile_pool(name="psum_h", bufs=1, space="PSUM") as ph:
                    bvb = ph.tile([128, C], F32, tag="bvb")
                    nc.tensor.matmul(
                        bvb[:], ones_row_bf[:], bv_row_bf[:], start=True, stop=True
                    )
                    nc.vector.tensor_copy(bv_bcast[:], bvb[:])

                    def qk_proj(q):
                        qsl = slice(q * TB, (q + 1) * TB)
                        ps = ph.tile([128, TB], F32, tag="qkp", bufs=2, name=f"qkp_{q}")
                        for ci in range(NCH):
                            nc.tensor.matmul(
                                ps[:],
                                wqkT[:, ci, :],
                                x_bf[:, ci, qsl],
                                start=(ci == 0),
                                stop=(ci == NCH - 1),
                            )
                        nc.vector.tensor_scalar_add(qk[:, qsl], ps[:], bqk[:, 0:1])
                        nc.sync.dma_start(out=kq[0:CQ, qsl], in_=qk[CQ:128, qsl])
                        nc.sync.dma_start(out=kq[CQ:128, qsl], in_=qk[0:CQ, qsl])

                    qk_proj(0)
                    for q in range(NTB):
                        for j in range(4 * q, 4 * q + 4):
                            psv = ph.tile([128, C], F32, tag="vp", bufs=2, name=f"vp_{j}")
                            for ci in range(NCH):
                                nc.tensor.matmul(
                                    psv[:],
                                    x_bf[:, ci, j * 128:(j + 1) * 128],
                                    wvT[:, ci, :],
                                    start=(ci == 0),
                                    stop=(ci == NCH - 1),
                                )
                            nc.vector.tensor_add(vT[:, j, :], psv[:], bv_bcast[:])
                        emit_scores(0, 2 * q)
                        emit_scores(0, 2 * q + 1)
                        if q + 1 < NTB:
                            qk_proj(q + 1)

                with (
                    tc.tile_pool(name="ps_av", bufs=1, space="PSUM") as ps_av,
                    tc.tile_pool(name="ps_dn", bufs=2, space="PSUM") as ps_dn,
                    tc.tile_pool(name="small", bufs=2) as small,
                    tc.tile_pool(name="outp", bufs=2) as outp,
                ):
                    avs = {}
                    dns = {}
                    rbs = {}
                    NBLK = NPAIR // 2

                    def start_tb(tb):
                        avs[tb] = [
                            ps_av.tile(
                                [128, TB], F32, tag=f"av{ck}", name=f"av{ck}_{tb}"
                            )
                            for ck in range(NCC)
                        ]
                        dns[tb] = ps_dn.tile([128, TB], F32, tag="dn", name=f"dn_{tb}")

                    def consume_pair(tb, jp):
                        etp = etp_of[(tb, jp)]
                        if jp < NBLK:
                            # one denominator batch per early pair, right
                            # after the score matmuls (which already paid the
                            # PE row-config switch): 4 col-tiled concurrent
                            # ones-matmuls.  Batch 0 carries start=True on
                            # all four tiles: the whole-bank has_written
                            # clears complete before the first drain write
                            # lands, so the concurrent clears are safe.
                            b = jp
                            for k in range(4):
                                e = etp_of[(tb, 2 * b + k // 2)]
                                nc.tensor.matmul(
                                    dns[tb][32 * k:32 * k + 1, :],
                                    ones_col[:],
                                    e[:, k % 2, :],
                                    start=(b == 0),
                                    stop=(b == NBLK - 1),
                                    tile_position=(0, 32 * k),
                                    skip_group_check=True,
                                )
                            if b == NBLK - 1:
                                # pull the partials on DVE in the shadow of
                                # the AV matmuls
                                for k in range(4):
                                    nc.vector.tensor_copy(
                                        d4sb[32 * k:32 * k + 1, :],
                                        dns[tb][32 * k:32 * k + 1, :],
                                    )
                        if jp == 5:
                            # sum + broadcast the 4 partials in one bf16
                            # matmul; reciprocal on DVE hides under the AVs
                            rbp = ps_dn.tile(
                                [128, TB], F32, tag="dn", name=f"rbp_{tb}"
                            )
                            nc.tensor.matmul(
                                rbp[:], ones128b[:], d4sb[:], start=True, stop=True
                            )
                            rb = small.tile([128, TB], F32, tag="rb", name=f"rb_{tb}")
                            nc.vector.reciprocal_approx_fast(rb[:], rbp[:])
                            rbs[tb] = rb

                        if jp < NPAIR - 1:
                            for idx in (0, 1):
                                j = 2 * jp + idx
                                for ck in range(NCC):
                                    nc.tensor.matmul(
                                        avs[tb][ck][:],
                                        vT[:, j, ck * 128:(ck + 1) * 128],
                                        etp[:, idx, :],
                                        start=(j == 0),
                                        stop=False,
                                    )
                        else:
                            # final pair: channel-major so each output chunk
                            # finishes early and its normalize + store starts
                            # while the remaining chunks still accumulate
                            tsl = slice(tb * TB, (tb + 1) * TB)
                            j0, j1 = 2 * jp, 2 * jp + 1
                            for ck in range(NCC):
                                nc.tensor.matmul(
                                    avs[tb][ck][:],
                                    vT[:, j0, ck * 128:(ck + 1) * 128],
                                    etp[:, 0, :],
                                    start=False,
                                    stop=False,
                                )
                                nc.tensor.matmul(
                                    avs[tb][ck][:],
                                    vT[:, j1, ck * 128:(ck + 1) * 128],
                                    etp[:, 1, :],
                                    start=False,
                                    stop=True,
                                )
                                ot = outp.tile(
                                    [128, TB], F32, tag=f"ot{ck}", name=f"ot{ck}_{tb}"
                                )
                                nc.vector.tensor_mul(ot[:], avs[tb][ck][:], rbs[tb][:])
                                nc.sync.dma_start(
                                    out=out_d[ck * 128:(ck + 1) * 128, tsl], in_=ot[:]
                                )
                            if tb + 1 < NTB:
                                start_tb(tb + 1)

                    pairs = [(tb, jp) for tb in range(NTB) for jp in range(NPAIR)]
                    start_tb(0)
                    for i, (tb, jp) in enumerate(pairs):
                        if i + LOOKAHEAD < len(pairs):
                            emit_scores(*pairs[i + LOOKAHEAD])
                        consume_pair(tb, jp)

    nc.compile()
    return nc


_PROGRAM = None


def _get_program() -> bass.Bass:
    global _PROGRAM
    if _PROGRAM is None:
        _PROGRAM = _build_program()
    return _PROGRAM


def kernel(**inputs: np.ndarray) -> np.ndarray:
    x = np.ascontiguousarray(np.asarray(inputs["x"], dtype=np.float32))
    wq = np.ascontiguousarray(np.asarray(inputs["Wq"], dtype=np.float32))
    bq = np.ascontiguousarray(np.asarray(inputs["bq"], dtype=np.float32)).reshape(CQ, 1)
    wk = np.ascontiguousarray(np.asarray(inputs["Wk"], dtype=np.float32))
    bk = np.ascontiguousarray(np.asarray(inputs["bk"], dtype=np.float32)).reshape(CQ, 1)
    wv = np.ascontiguousarray(np.asarray(inputs["Wv"], dtype=np.float32))
    bv = np.ascontiguousarray(np.asarray(inputs["bv"], dtype=np.float32)).reshape(1, C)

    ident = np.eye(128, dtype=np.float32)
    nc = _get_program()
    in_maps = [
        {
            "x": np.ascontiguousarray(x[b]),
            "Wq": wq,
            "bq": bq,
            "Wk": wk,
            "bk": bk,
            "Wv": wv,
            "bv": bv,
            "ident": ident,
        }
        for b in range(NCORES)
    ]
    res = run_bass_kernel_spmd(nc, in_maps, list(range(NCORES)))
    out = np.stack([res.results[b]["out"] for b in range(NCORES)], axis=0)
    return out.astype(np.float32)


if __name__ == "__main__":
    import reference

    inputs = {k: np.asarray(v) for k, v in reference.setup_inputs().items()}
    expected = np.asarray(reference.reference(**inputs))
    actual = kernel(**inputs)
    rel = np.linalg.norm(actual - expected) / np.linalg.norm(expected)
    print("Relative error:", rel)
